# revision 17
# baseline (speedup 1.0000x reference)
"""RGCN (segment_reduce) Trainium2 kernel — 8 NeuronCores, full inputs in/out.

Per core = one dst-shard of N/8 nodes and NROW/8 data rows.
  - Edges CPU-binned by (dst-node-block, relation, src-half); per 128-edge
    tile: dma_gather rows [h | norm_s] (768B) from a replicated DRAM table,
    build a scaled one-hot on DVE (iota == dstloc)*ns, and aggregate with one
    PE matmul per tile into PSUM (no scatter DMAs anywhere).
  - Degrees/norms computed on device via one-hot colsum matmuls (bf16).
  - Per-layer tables rebuilt per shard and AllGathered.
  - Row stage (gather + masked mean + MLP) reuses the same machinery; masked
    pairs are dropped during CPU index binning.
"""
import sys
import types
from contextlib import ExitStack

import numpy as np

if "antenv" not in sys.modules:
    try:
        import antenv  # noqa: F401
    except ImportError:
        _antenv = types.ModuleType("antenv")
        _antenv.__path__ = []
        sys.modules["antenv"] = _antenv

import concourse.bass as bass  # noqa: E402
import concourse.bacc as bacc  # noqa: E402
import concourse.tile as tile  # noqa: E402
from concourse import mybir  # noqa: E402
from concourse.masks import make_identity  # noqa: E402
import concourse.bass_utils as bass_utils  # noqa: E402

_DGE_ARGS = [
    "--dge-levels=scalar_dynamic_offset",
    "--dge-levels=vector_dynamic_offsets",
    "--dge-levels=dst_reduce",
]
if not getattr(bass_utils, "_dge_patched", False):
    _orig_run_command = bass_utils.run_command

    def _run_command_dge(argv, **kwargs):
        if argv and "walrus_driver" in str(argv[0]) and "--pass" in argv:
            argv = list(argv) + [a for a in _DGE_ARGS if a not in argv]
        return _orig_run_command(argv, **kwargs)

    bass_utils.run_command = _run_command_dge
    bass_utils._dge_patched = True

F32 = mybir.dt.float32
BF16 = mybir.dt.bfloat16
I16 = mybir.dt.int16
AF = mybir.ActivationFunctionType
ALU = mybir.AluOpType

N_CORES = 8
P = 128
CHUNK_TILES = 8
CHUNK = CHUNK_TILES * P
SPLIT = 32768  # int16 gather-index limit


def _ceil(a, b):
    return -(-a // b)


class Struct:
    pass


# ---------------------------------------------------------------------------
# CPU-side binning (pure numpy)
# ---------------------------------------------------------------------------
def _bin_by_dst(payload, dst, shard, n_blk, half_split):
    """group by (core, dst block); payload split by < SPLIT if asked."""
    out = []
    for c in range(N_CORES):
        lo, hi = c * shard, (c + 1) * shard
        sel = (dst >= lo) & (dst < hi)
        ds = dst[sel] - lo
        ss = payload[sel]
        blk = ds // P
        order = np.argsort(blk, kind="stable")
        ds, ss, blk = ds[order], ss[order], blk[order]
        bounds = np.searchsorted(blk, np.arange(n_blk + 1))
        perblk = []
        for b in range(n_blk):
            sb = ss[bounds[b]:bounds[b + 1]]
            db = ds[bounds[b]:bounds[b + 1]] - b * P
            if half_split:
                m = sb < SPLIT
                perblk.append((sb[m], db[m], sb[~m] - SPLIT, db[~m]))
            else:
                perblk.append((sb, db, np.zeros(0, np.int64), np.zeros(0, np.int64)))
        out.append(perblk)
    return out


def _pack(groups_rc, n_blk, R, half_split):
    """groups_rc[r][c][b] -> common tiles + per-core src16/dstloc arrays."""
    nt = np.zeros((n_blk, R, 2), np.int64)
    for r in range(R):
        for c in range(N_CORES):
            for b in range(n_blk):
                g = groups_rc[r][c][b]
                nt[b, r, 0] = max(nt[b, r, 0], _ceil(len(g[0]), P))
                if half_split:
                    nt[b, r, 1] = max(nt[b, r, 1], _ceil(len(g[2]), P))
    tiles = []
    tmap = {}
    for b in range(n_blk):
        for r in range(R):
            tot = int(nt[b, r, 0] + nt[b, r, 1])
            k = 0
            for half in (0, 1):
                for j in range(int(nt[b, r, half])):
                    tmap[(b, r, half, j)] = len(tiles)
                    tiles.append((b, r, half, k == 0, k == tot - 1))
                    k += 1
    NT = len(tiles)
    src16 = np.zeros((N_CORES, NT, P), np.int16)
    dloc = np.full((N_CORES, NT, P), -1.0, np.float32)
    for c in range(N_CORES):
        for b in range(n_blk):
            for r in range(R):
                g = groups_rc[r][c][b]
                for half in (0, 1):
                    sarr = g[0] if half == 0 else g[2]
                    darr = g[1] if half == 0 else g[3]
                    for j in range(_ceil(len(sarr), P)):
                        t = tmap[(b, r, half, j)]
                        seg_s = sarr[j * P:(j + 1) * P]
                        seg_d = darr[j * P:(j + 1) * P]
                        src16[c, t, :len(seg_s)] = seg_s
                        dloc[c, t, :len(seg_d)] = seg_d
    return tiles, src16, dloc


def _chunks_of(tiles):
    lo = [i for i, t in enumerate(tiles) if t[2] == 0]
    hi = [i for i, t in enumerate(tiles) if t[2] == 1]
    chunks = []
    for half, stream in ((0, lo), (1, hi)):
        for i in range(0, len(stream), CHUNK_TILES):
            chunks.append((half, stream[i:i + CHUNK_TILES]))
    chunks.sort(key=lambda ch: min(ch[1]))
    slot = {}
    for ci, (_, tl) in enumerate(chunks):
        for j, t in enumerate(tl):
            slot[t] = (ci, j)
    return chunks, slot


def _wrap_idx(src16, chunks):
    ncore = src16.shape[0]
    out = np.zeros((ncore, 128, max(1, len(chunks)) * (CHUNK // 16)), np.int16)
    for ci, (_, tl) in enumerate(chunks):
        flat = np.zeros((ncore, CHUNK), np.int16)
        for j, t in enumerate(tl):
            flat[:, j * P:(j + 1) * P] = src16[:, t, :]
        out[:, :16, ci * 64:(ci + 1) * 64] = flat.reshape(
            ncore, CHUNK // 16, 16).transpose(0, 2, 1)
    out[:, 16:, :] = np.tile(out[:, :16, :], (1, 7, 1))
    return out


def prepare(inputs, cfg):
    s = Struct()
    s.cfg = cfg
    N, R, NROW, FK = cfg["N"], cfg["R"], cfg["NROW"], cfg["F"]
    shard, rshard = N // N_CORES, NROW // N_CORES
    n_blk, n_rblk = _ceil(shard, P), _ceil(rshard, P)
    s.shard, s.rshard, s.n_blk, s.n_rblk = shard, rshard, n_blk, n_rblk

    es = np.asarray(inputs["edges_src"]).astype(np.int64)
    ed = np.asarray(inputs["edges_dst"]).astype(np.int64)

    g_main = [_bin_by_dst(es[r], ed[r], shard, n_blk, True) for r in range(R)]
    s.tiles_e, src16_e, dloc_e = _pack(g_main, n_blk, R, True)
    s.chunks_e, s.slot_e = _chunks_of(s.tiles_e)
    s.idx_e = _wrap_idx(src16_e, s.chunks_e)

    g_src = [_bin_by_dst(es[r], es[r], shard, n_blk, False) for r in range(R)]
    s.tiles_s, _, sloc_s = _pack(g_src, n_blk, R, False)

    ridx = np.asarray(inputs["row_idx"]).astype(np.int64)
    rmask = np.asarray(inputs["row_mask"]).astype(bool)
    g_row = [[]]
    for c in range(N_CORES):
        lo = c * rshard
        rows = []
        for bb in range(n_rblk):
            i0 = lo + bb * P
            i1 = min(i0 + P, lo + rshard)
            ii, jj = np.nonzero(rmask[i0:i1])
            srcs = ridx[i0:i1][ii, jj]
            m = srcs < SPLIT
            rows.append((srcs[m], ii[m], srcs[~m] - SPLIT, ii[~m]))
        g_row[0].append(rows)
    s.tiles_r, src16_r, dloc_r = _pack(g_row, n_rblk, 1, True)
    s.chunks_r, s.slot_r = _chunks_of(s.tiles_r)
    s.idx_r = _wrap_idx(src16_r, s.chunks_r)

    s.NT_e, s.NT_s, s.NT_r = len(s.tiles_e), len(s.tiles_s), len(s.tiles_r)
    s.dloc_e = dloc_e.transpose(0, 2, 1)   # [c, 128, NT]
    s.sloc_s = sloc_s.transpose(0, 2, 1)
    s.dloc_r = dloc_r.transpose(0, 2, 1)

    nf = np.asarray(inputs["node_feats"]).astype(np.float32)
    s.nf_shards = [nf[c * shard:(c + 1) * shard] for c in range(N_CORES)]
    mk = rmask.astype(np.float32)
    s.mask_shards = []
    for c in range(N_CORES):
        m = np.zeros((n_rblk * P, FK), np.float32)
        m[:rshard] = mk[c * rshard:(c + 1) * rshard]
        s.mask_shards.append(np.ascontiguousarray(
            m.reshape(n_rblk, P, FK).transpose(1, 0, 2)).reshape(128, -1))
    return s


# ---------------------------------------------------------------------------
# device program
# ---------------------------------------------------------------------------
def build_program(s):
    cfg = s.cfg
    N, R, FK = cfg["N"], cfg["R"], cfg["F"]
    IN_D, HID, NCLS = cfg["IN"], cfg["HID"], cfg["NCLS"]
    TW = 192
    n_blk, n_rblk, shard, rshard = s.n_blk, s.n_rblk, s.shard, s.rshard
    NQ = 4
    NDEG = n_blk * R
    ndeg_tiles = _ceil(2 * NDEG, 128)

    nc = bacc.Bacc("TRN2", target_bir_lowering=False, debug=False,
                   num_devices=N_CORES, num_swdge_queues=NQ)
    dp = nc.declare_dram_parameter
    t_nf = dp("nf", [n_blk * P, IN_D], F32, isOutput=False)
    t_Win = dp("W_in", [IN_D, HID], F32, isOutput=False)
    t_bin = dp("b_in", [1, HID], F32, isOutput=False)
    t_W1 = dp("W1", [R * HID, HID], F32, isOutput=False)
    t_b1 = dp("b1", [R, HID], F32, isOutput=False)
    t_W2 = dp("W2", [R * HID, HID], F32, isOutput=False)
    t_b2 = dp("b2", [R, HID], F32, isOutput=False)
    t_Wm1 = dp("Wm1", [HID, HID], F32, isOutput=False)
    t_bm1 = dp("bm1", [HID, 1], F32, isOutput=False)
    t_Wm2 = dp("Wm2", [HID, HID], F32, isOutput=False)
    t_bm2 = dp("bm2", [HID, 1], F32, isOutput=False)
    t_Wm3 = dp("Wm3", [HID, NCLS], F32, isOutput=False)
    t_bm3 = dp("bm3", [NCLS, 1], F32, isOutput=False)
    t_idx_e = dp("idx_e", list(s.idx_e.shape[1:]), I16, isOutput=False)
    t_idx_r = dp("idx_r", list(s.idx_r.shape[1:]), I16, isOutput=False)
    t_dle = dp("dloc_e", [128, max(1, s.NT_e)], F32, isOutput=False)
    t_dls = dp("sloc_s", [128, max(1, s.NT_s)], F32, isOutput=False)
    t_dlr = dp("dloc_r", [128, max(1, s.NT_r)], F32, isOutput=False)
    t_mask = dp("mask", [128, n_rblk * FK], F32, isOutput=False)
    t_selR = dp("selR", [16, R * 128], F32, isOutput=False)
    t_selB = dp("selB", [max(64, n_rblk), n_rblk * 128], F32, isOutput=False)
    t_out = dp("out", [NCLS, rshard], F32, isOutput=True)

    T = [nc.dram_tensor(f"T{i}", [N, TW], F32) for i in range(3)]
    Tsh = [nc.dram_tensor(f"T{i}sh", [shard, TW], F32) for i in range(3)]

    with tile.TileContext(nc) as tc, ExitStack() as top:
        kp = top.enter_context(tc.tile_pool(name="const", bufs=1))
        wp = top.enter_context(tc.tile_pool(name="weights", bufs=1))
        mp = top.enter_context(tc.tile_pool(name="meta", bufs=1))
        sb = top.enter_context(tc.tile_pool(name="sbwork", bufs=4))
        ttp = top.enter_context(tc.tile_pool(name="ttile", bufs=4))

        iota = kp.tile([128, 128], F32)
        nc.gpsimd.iota(iota[:], pattern=[[1, 128]], base=0, channel_multiplier=0,
                       allow_small_or_imprecise_dtypes=True)
        iota16 = kp.tile([128, 128], BF16)
        nc.gpsimd.iota(iota16[:], pattern=[[1, 128]], base=0, channel_multiplier=0,
                       allow_small_or_imprecise_dtypes=True)
        ident = kp.tile([128, 128], F32)
        make_identity(nc, ident[:])
        ones_col16 = kp.tile([128, 1], BF16)
        nc.vector.memset(ones_col16[:], 1.0)
        ones1 = kp.tile([1, 128], F32)
        nc.vector.memset(ones1[:], 1.0)
        onesR = kp.tile([R, 1], F32)
        nc.vector.memset(onesR[:], 1.0)
        selR = kp.tile([16, R * 128], F32)
        nc.sync.dma_start(out=selR[:], in_=t_selR[:])

        W1sb = wp.tile([128, R * HID], F32)
        W2sb = wp.tile([128, R * HID], F32)
        for r in range(R):
            nc.sync.dma_start(out=W1sb[:HID, r * HID:(r + 1) * HID],
                              in_=t_W1[r * HID:(r + 1) * HID, :])
            nc.sync.dma_start(out=W2sb[:HID, r * HID:(r + 1) * HID],
                              in_=t_W2[r * HID:(r + 1) * HID, :])
        Winsb = wp.tile([IN_D, HID], F32)
        nc.sync.dma_start(out=Winsb[:], in_=t_Win[:])
        Wm1sb = wp.tile([HID, HID], F32)
        nc.sync.dma_start(out=Wm1sb[:], in_=t_Wm1[:])
        Wm2sb = wp.tile([HID, HID], F32)
        nc.sync.dma_start(out=Wm2sb[:], in_=t_Wm2[:])
        Wm3sb = wp.tile([HID, NCLS], F32)
        nc.sync.dma_start(out=Wm3sb[:], in_=t_Wm3[:])
        b1sb = wp.tile([R, HID], F32)
        nc.sync.dma_start(out=b1sb[:], in_=t_b1[:])
        b2sb = wp.tile([R, HID], F32)
        nc.sync.dma_start(out=b2sb[:], in_=t_b2[:])
        binsb = wp.tile([1, HID], F32)
        nc.sync.dma_start(out=binsb[:], in_=t_bin[:])
        bm1sb = wp.tile([HID, 1], F32)
        nc.sync.dma_start(out=bm1sb[:], in_=t_bm1[:])
        bm2sb = wp.tile([HID, 1], F32)
        nc.sync.dma_start(out=bm2sb[:], in_=t_bm2[:])
        bm3sb = wp.tile([NCLS, 1], F32)
        nc.sync.dma_start(out=bm3sb[:], in_=t_bm3[:])

        dle = mp.tile([128, max(1, s.NT_e)], F32)
        nc.sync.dma_start(out=dle[:], in_=t_dle[:])
        dls = mp.tile([128, max(1, s.NT_s)], F32)
        nc.sync.dma_start(out=dls[:], in_=t_dls[:])
        dlr = mp.tile([128, max(1, s.NT_r)], F32)
        nc.sync.dma_start(out=dlr[:], in_=t_dlr[:])
        masksb = mp.tile([128, n_rblk * FK], F32)
        nc.sync.dma_start(out=masksb[:], in_=t_mask[:])

        # ---- replicated biases ------------------------------------------
        binrep = wp.tile([128, HID], F32)
        bsum1 = wp.tile([HID, 1], F32)
        bsum2 = wp.tile([HID, 1], F32)
        with tc.tile_pool(name="ps_init", bufs=1, space="PSUM") as pk:
            ps = pk.tile([128, HID], F32, tag="a")
            nc.tensor.matmul(ps[:], lhsT=ones1[:], rhs=binsb[:], start=True, stop=True)
            nc.vector.tensor_copy(binrep[:], ps[:])
            ps2 = pk.tile([HID, 1], F32, tag="b")
            nc.tensor.matmul(ps2[:], lhsT=b1sb[:], rhs=onesR[:], start=True, stop=True)
            nc.vector.tensor_copy(bsum1[:], ps2[:])
            ps3 = pk.tile([HID, 1], F32, tag="c")
            nc.tensor.matmul(ps3[:], lhsT=b2sb[:], rhs=onesR[:], start=True, stop=True)
            nc.vector.tensor_copy(bsum2[:], ps3[:])

        # ---- phase 1: h0 -------------------------------------------------
        h0sb = mp.tile([128, n_blk * HID], F32)
        with tc.tile_pool(name="ps_h0", bufs=2, space="PSUM") as pp:
            for b in range(n_blk):
                rows = min(P, shard - b * P)
                xt = sb.tile([128, IN_D], F32, tag="xt")
                if rows < P:
                    nc.vector.memset(xt[:], 0.0)
                nc.sync.dma_start(out=xt[:rows, :], in_=t_nf[b * P:b * P + rows, :])
                pst = pp.tile([IN_D, 128], F32, tag="xT")
                nc.tensor.transpose(pst[:], xt[:, :IN_D], ident[:])
                xT = sb.tile([IN_D, 128], F32, tag="xTs")
                nc.vector.tensor_copy(xT[:], pst[:])
                psh = pp.tile([128, HID], F32, tag="h0")
                nc.tensor.matmul(psh[:], lhsT=xT[:], rhs=Winsb[:], start=True,
                                 stop=True)
                tmp = sb.tile([128, HID], F32, tag="h0t")
                nc.vector.tensor_tensor(out=tmp[:], in0=psh[:], in1=binrep[:],
                                        op=ALU.add)
                nc.scalar.activation(h0sb[:, b * HID:(b + 1) * HID], tmp[:], AF.Relu)

        # ---- phase 2: degrees -> norms ----------------------------------
        # per block: psum [128, 16] cols 0:8 deg_in(r), 8:16 deg_out(r)
        e_byblk = {}
        for ti, (b, r, half, first, last) in enumerate(s.tiles_e):
            e_byblk.setdefault(b, []).append((ti, r, last))
        s_byblk = {}
        for ti, (b, r, half, first, last) in enumerate(s.tiles_s):
            s_byblk.setdefault(b, []).append((ti, r, last))
        nscols = mp.tile([128, n_blk * 16], F32)   # n-part norms per block
        normsT = mp.tile([16, n_blk * 128], F32)   # free-dim norms per block
        with (
            tc.tile_pool(name="ps_deg", bufs=2, space="PSUM") as dgp,
            tc.tile_pool(name="ps_degT", bufs=2, space="PSUM") as dtp,
        ):
            for b in range(n_blk):
                degblk = dgp.tile([128, 16], F32, tag="dg")
                nc.vector.memset(degblk[:], 0.0)
                dstarted = set()
                for col_base, byblk, dl in ((0, e_byblk, dle), (8, s_byblk, dls)):
                    for (ti, r, last) in byblk.get(b, []):
                        oh = sb.tile([128, 128], BF16, tag="ohd")
                        nc.vector.tensor_scalar(
                            out=oh[:], in0=iota16[:], scalar1=dl[:, ti:ti + 1],
                            scalar2=None, op0=ALU.is_equal)
                        nc.tensor.matmul(
                            degblk[:, col_base + r:col_base + r + 1],
                            lhsT=oh[:], rhs=ones_col16[:],
                            start=(col_base + r) not in dstarted, stop=last)
                        dstarted.add(col_base + r)
                tmp = sb.tile([128, 16], F32, tag="degt")
                nc.vector.tensor_scalar(out=tmp[:], in0=degblk[:], scalar1=1.0,
                                        scalar2=None, op0=ALU.max)
                tmp2 = sb.tile([128, 16], F32, tag="degt2")
                nc.vector.reciprocal(tmp2[:], tmp[:])
                nc.scalar.activation(nscols[:, b * 16:(b + 1) * 16], tmp2[:],
                                     AF.Sqrt)
                pst = dtp.tile([16, 128], F32, tag="dT")
                nc.tensor.transpose(pst[:], nscols[:, b * 16:(b + 1) * 16],
                                    ident[:])
                nc.vector.tensor_copy(normsT[:, b * 128:(b + 1) * 128], pst[:])

        def write_table(l, b, fill_h):
            rows = min(P, shard - b * P)
            tt = ttp.tile([128, TW], F32, tag="tt")
            nc.vector.memset(tt[:, HID + R:TW], 0.0)
            fill_h(tt)
            nc.vector.tensor_copy(tt[:, HID:HID + R],
                                  nscols[:, b * 16 + 8:b * 16 + 8 + R])
            nc.sync.dma_start(out=Tsh[l][b * P:b * P + rows, :], in_=tt[:rows, :])

        def allgather(l):
            nc.gpsimd.collective_compute(
                "AllGather", ALU.bypass,
                replica_groups=[list(range(N_CORES))],
                ins=[Tsh[l][:]], outs=[T[l][:]])

        for b in range(n_blk):
            write_table(0, b, lambda tt, b=b: nc.vector.tensor_copy(
                tt[:, 0:HID], h0sb[:, b * HID:(b + 1) * HID]))
        allgather(0)

        # ---- phases 3&4: the two RGCN layers ----------------------------
        gp = top.enter_context(tc.tile_pool(name="gather", bufs=10))
        ip = top.enter_context(tc.tile_pool(name="idxt", bufs=8))

        blk_maxr = {}
        for (b, r, half, first, last) in s.tiles_e:
            blk_maxr[b] = max(blk_maxr.get(b, -1), r)

        def run_layer(l):
            Wsb = W1sb if l == 0 else W2sb
            bsum = bsum1 if l == 0 else bsum2
            with (
                tc.tile_pool(name=f"psx{l}", bufs=2, space="PSUM") as psxp,
                tc.tile_pool(name=f"ps2{l}", bufs=1, space="PSUM") as ps2p,
                tc.tile_pool(name=f"aux{l}", bufs=2, space="PSUM") as auxp,
            ):
                gtiles = {}
                for ci, (half, tl) in enumerate(s.chunks_e):
                    it = ip.tile([128, 64], I16, tag="ie")
                    nc.sync.dma_start(out=it[:], in_=t_idx_e[:, ci * 64:(ci + 1) * 64])
                    g = gp.tile([128, CHUNK_TILES, TW], F32, tag="ge")
                    src = T[l][0:SPLIT, :] if half == 0 else T[l][SPLIT:N, :]
                    nc.gpsimd.dma_gather(
                        out_ap=g[:], in_ap=src, idxs_ap=it[:],
                        num_idxs=CHUNK, num_idxs_reg=CHUNK, elem_size=TW,
                        queue_num=ci % NQ)
                    gtiles[ci] = g

                def flush_block(b, psx, started):
                    ps2 = ps2p.tile([128, 128], F32, tag="p2")
                    maxr = blk_maxr[b]
                    any_r = False
                    for r in range(R):
                        if (b, r) not in started:
                            continue
                        pst = auxp.tile([128, 128], F32, tag="ax")
                        nc.tensor.matmul(
                            pst[:], lhsT=selR[:, r * 128:(r + 1) * 128],
                            rhs=normsT[:, b * 128:(b + 1) * 128],
                            start=True, stop=True)
                        ndrep = sb.tile([128, 128], F32, tag="ndr")
                        nc.vector.tensor_copy(ndrep[:], pst[:])
                        xs = sb.tile([128, 128], F32, tag="xs")
                        nc.vector.tensor_tensor(out=xs[:], in0=psx[:, r, :],
                                                in1=ndrep[:], op=ALU.mult)
                        nc.tensor.matmul(ps2[:], lhsT=Wsb[:HID, r * HID:(r + 1) * HID],
                                         rhs=xs[:], start=not any_r,
                                         stop=(r == maxr))
                        any_r = True
                    hsb = sb.tile([128, 128], F32, tag="hsb")
                    if l == 0:
                        nc.scalar.activation(hsb[:], ps2[:], AF.Relu,
                                             bias=bsum[:])
                    else:
                        nc.vector.tensor_scalar(
                            out=hsb[:], in0=ps2[:], scalar1=bsum[:],
                            scalar2=None, op0=ALU.add)
                    pst2 = auxp.tile([128, 128], F32, tag="ax")
                    nc.tensor.transpose(pst2[:], hsb[:], ident[:])
                    write_table(l + 1, b, lambda tt: nc.vector.tensor_copy(
                        tt[:, 0:HID], pst2[:]))

                cur_blk, psx, started = -1, None, set()
                for ti, (b, r, half, first, last) in enumerate(s.tiles_e):
                    if b != cur_blk:
                        if cur_blk >= 0:
                            flush_block(cur_blk, psx, started)
                        cur_blk = b
                        psx = psxp.tile([128, R, 128], F32, tag="psx")
                        started = set()
                    ci, j = s.slot_e[ti]
                    g = gtiles[ci]
                    mg = sb.tile([128, 128], BF16, tag="mg")
                    nc.vector.tensor_scalar(
                        out=mg[:], in0=g[:, j, 0:HID],
                        scalar1=g[:, j, HID + r:HID + r + 1],
                        scalar2=None, op0=ALU.mult)
                    oh = sb.tile([128, 128], BF16, tag="ohm")
                    nc.vector.tensor_scalar(
                        out=oh[:], in0=iota16[:], scalar1=dle[:, ti:ti + 1],
                        scalar2=None, op0=ALU.is_equal)
                    nc.tensor.matmul(psx[:, r, :], lhsT=mg[:], rhs=oh[:],
                                     start=(b, r) not in started, stop=last)
                    started.add((b, r))
                if cur_blk >= 0:
                    flush_block(cur_blk, psx, started)
            allgather(l + 1)

        run_layer(0)
        run_layer(1)

        # ---- phase 5: rows + MLP ----------------------------------------
        NR64 = max(64, n_rblk)
        cnt = mp.tile([128, n_rblk], F32)
        nc.vector.reduce_sum(
            out=cnt[:],
            in_=masksb[:].rearrange("p (b f) -> p b f", f=FK),
            axis=mybir.AxisListType.X)
        cnt2 = mp.tile([128, NR64], F32)
        nc.vector.memset(cnt2[:], 1.0)
        nc.vector.tensor_scalar(out=cnt2[:, :n_rblk], in0=cnt[:], scalar1=1.0,
                                scalar2=None, op0=ALU.max)
        rc = mp.tile([128, NR64], F32)
        nc.vector.reciprocal(rc[:], cnt2[:])
        rcT = mp.tile([NR64, 128], F32)
        with tc.tile_pool(name="ps_rc", bufs=1, space="PSUM") as pp:
            pst = pp.tile([NR64, 128], F32, tag="rcT")
            nc.tensor.transpose(pst[:], rc[:], ident[:])
            nc.vector.tensor_copy(rcT[:], pst[:])
        selB = mp.tile([NR64, n_rblk * 128], F32)
        nc.sync.dma_start(out=selB[:], in_=t_selB[:])

        with (
            tc.tile_pool(name="psr", bufs=2, space="PSUM") as psrp,
            tc.tile_pool(name="psm", bufs=2, space="PSUM") as psmp,
            tc.tile_pool(name="auxr", bufs=2, space="PSUM") as auxp,
        ):
            gtiles = {}
            for ci, (half, tl) in enumerate(s.chunks_r):
                it = ip.tile([128, 64], I16, tag="ir")
                nc.sync.dma_start(out=it[:], in_=t_idx_r[:, ci * 64:(ci + 1) * 64])
                g = gp.tile([128, CHUNK_TILES, TW], F32, tag="ge")
                src = T[2][0:SPLIT, :] if half == 0 else T[2][SPLIT:N, :]
                nc.gpsimd.dma_gather(
                    out_ap=g[:], in_ap=src, idxs_ap=it[:],
                    num_idxs=CHUNK, num_idxs_reg=CHUNK, elem_size=TW,
                    queue_num=ci % NQ)
                gtiles[ci] = g

            def flush_rblock(bb, psr):
                pst = auxp.tile([128, 128], F32, tag="axr")
                nc.tensor.matmul(pst[:], lhsT=selB[:, bb * 128:(bb + 1) * 128],
                                 rhs=rcT[:], start=True, stop=True)
                rrep = sb.tile([128, 128], F32, tag="rrep")
                nc.vector.tensor_copy(rrep[:], pst[:])
                xr = sb.tile([128, 128], F32, tag="xr")
                nc.vector.tensor_tensor(out=xr[:], in0=psr[:], in1=rrep[:],
                                        op=ALU.mult)
                pm = psmp.tile([128, 128], F32, tag="pm")
                nc.tensor.matmul(pm[:], lhsT=Wm1sb[:], rhs=xr[:], start=True,
                                 stop=True)
                a1 = sb.tile([128, 128], F32, tag="a1")
                nc.scalar.activation(a1[:], pm[:], AF.Relu, bias=bm1sb[:])
                pm2 = psmp.tile([128, 128], F32, tag="pm")
                nc.tensor.matmul(pm2[:], lhsT=Wm2sb[:], rhs=a1[:], start=True,
                                 stop=True)
                a2 = sb.tile([128, 128], F32, tag="a2")
                nc.scalar.activation(a2[:], pm2[:], AF.Relu, bias=bm2sb[:])
                pm3 = psmp.tile([NCLS, 128], F32, tag="pm")
                nc.tensor.matmul(pm3[:], lhsT=Wm3sb[:], rhs=a2[:], start=True,
                                 stop=True)
                ot = sb.tile([NCLS, 128], F32, tag="ot")
                nc.vector.tensor_scalar(out=ot[:], in0=pm3[:], scalar1=bm3sb[:],
                                        scalar2=None, op0=ALU.add)
                cols = min(P, rshard - bb * P)
                nc.sync.dma_start(out=t_out[:, bb * P:bb * P + cols],
                                  in_=ot[:, :cols])

            r_byblk = {}
            for ti, (bb, r0, half, first, last) in enumerate(s.tiles_r):
                r_byblk.setdefault(bb, []).append((ti, last))
            for bb in range(n_rblk):
                psr = psrp.tile([128, 128], F32, tag="psrT")
                nc.vector.memset(psr[:], 0.0)
                rstarted = False
                for (ti, last) in r_byblk.get(bb, []):
                    ci, j = s.slot_r[ti]
                    g = gtiles[ci]
                    mg = sb.tile([128, 128], BF16, tag="mg")
                    nc.vector.tensor_copy(mg[:], g[:, j, 0:HID])
                    oh = sb.tile([128, 128], BF16, tag="ohm")
                    nc.vector.tensor_scalar(
                        out=oh[:], in0=iota16[:], scalar1=dlr[:, ti:ti + 1],
                        scalar2=None, op0=ALU.is_equal)
                    nc.tensor.matmul(psr[:], lhsT=mg[:], rhs=oh[:],
                                     start=not rstarted, stop=last)
                    rstarted = True
                flush_rblock(bb, psr)

    nc.compile()
    return nc


# ---------------------------------------------------------------------------
# entry point
# ---------------------------------------------------------------------------
def _selR(cfg):
    R = cfg["R"]
    a = np.zeros((16, R * 128), np.float32)
    for r in range(R):
        a[r, r * 128:(r + 1) * 128] = 1.0
    return a


def _selB(s):
    a = np.zeros((max(64, s.n_rblk), s.n_rblk * 128), np.float32)
    for bb in range(s.n_rblk):
        a[bb, bb * 128:(bb + 1) * 128] = 1.0
    return a


def run(inputs, cfg):
    s = prepare(inputs, cfg)
    nc = build_program(s)
    in_maps = []
    for c in range(N_CORES):
        nfp = np.zeros((s.n_blk * P, cfg["IN"]), np.float32)
        nfp[:s.shard] = s.nf_shards[c]
        m = {
            "nf": nfp,
            "W_in": np.asarray(inputs["W_in"], np.float32),
            "b_in": np.asarray(inputs["b_in"], np.float32).reshape(1, -1),
            "W1": np.asarray(inputs["W1"], np.float32).reshape(-1, cfg["HID"]),
            "b1": np.asarray(inputs["b1"], np.float32),
            "W2": np.asarray(inputs["W2"], np.float32).reshape(-1, cfg["HID"]),
            "b2": np.asarray(inputs["b2"], np.float32),
            "Wm1": np.asarray(inputs["Wm1"], np.float32),
            "bm1": np.asarray(inputs["bm1"], np.float32).reshape(-1, 1),
            "Wm2": np.asarray(inputs["Wm2"], np.float32),
            "bm2": np.asarray(inputs["bm2"], np.float32).reshape(-1, 1),
            "Wm3": np.asarray(inputs["Wm3"], np.float32),
            "bm3": np.asarray(inputs["bm3"], np.float32).reshape(-1, 1),
            "idx_e": np.ascontiguousarray(s.idx_e[c]),
            "idx_r": np.ascontiguousarray(s.idx_r[c]),
            "dloc_e": np.ascontiguousarray(s.dloc_e[c]),
            "sloc_s": np.ascontiguousarray(s.sloc_s[c]),
            "dloc_r": np.ascontiguousarray(s.dloc_r[c]),
            "mask": np.ascontiguousarray(s.mask_shards[c]),
            "selR": _selR(cfg),
            "selB": _selB(s),
        }
        in_maps.append(m)
    res = bass_utils.run_bass_kernel_spmd(nc, in_maps,
                                          core_ids=list(range(N_CORES)))
    out = np.concatenate(
        [res.results[c]["out"][:, :s.rshard].T for c in range(N_CORES)], axis=0)
    return out.astype(np.float32), s, nc, in_maps


def kernel(node_feats, edges_src, edges_dst, row_idx, row_mask,
           W_in, b_in, W1, b1, W2, b2, Wm1, bm1, Wm2, bm2, Wm3, bm3):
    cfg = dict(N=38000, R=8, NROW=60000, F=19, IN=64, HID=128, NCLS=10)
    inputs = dict(node_feats=node_feats, edges_src=edges_src,
                  edges_dst=edges_dst, row_idx=row_idx, row_mask=row_mask,
                  W_in=W_in, b_in=b_in, W1=W1, b1=b1, W2=W2, b2=b2,
                  Wm1=Wm1, bm1=bm1, Wm2=Wm2, bm2=bm2, Wm3=Wm3, bm3=bm3)
    out, _, _, _ = run(inputs, cfg)
    return out


# revision 18
# speedup vs baseline: 1.1623x; 1.1623x over previous
"""RGCN (segment_reduce) Trainium2 kernel — 8 NeuronCores, full inputs in/out.

Per core = one dst-shard of N/8 nodes and NROW/8 data rows.
  - Edges CPU-binned by (dst-node-block, relation, src-half); per 128-edge
    tile: dma_gather rows [h | norm_s] (768B) from a replicated DRAM table,
    build a scaled one-hot on DVE (iota == dstloc)*ns, and aggregate with one
    PE matmul per tile into PSUM (no scatter DMAs anywhere).
  - Degrees/norms computed on device via one-hot colsum matmuls (bf16).
  - Per-layer tables rebuilt per shard and AllGathered.
  - Row stage (gather + masked mean + MLP) reuses the same machinery; masked
    pairs are dropped during CPU index binning.
"""
import sys
import types
from contextlib import ExitStack

import numpy as np

if "antenv" not in sys.modules:
    try:
        import antenv  # noqa: F401
    except ImportError:
        _antenv = types.ModuleType("antenv")
        _antenv.__path__ = []
        sys.modules["antenv"] = _antenv

import concourse.bass as bass  # noqa: E402
import concourse.bacc as bacc  # noqa: E402
import concourse.tile as tile  # noqa: E402
from concourse import mybir  # noqa: E402
from concourse.masks import make_identity  # noqa: E402
import concourse.bass_utils as bass_utils  # noqa: E402

_DGE_ARGS = [
    "--dge-levels=scalar_dynamic_offset",
    "--dge-levels=vector_dynamic_offsets",
    "--dge-levels=dst_reduce",
]
if not getattr(bass_utils, "_dge_patched", False):
    _orig_run_command = bass_utils.run_command

    def _run_command_dge(argv, **kwargs):
        if argv and "walrus_driver" in str(argv[0]) and "--pass" in argv:
            argv = list(argv) + [a for a in _DGE_ARGS if a not in argv]
        return _orig_run_command(argv, **kwargs)

    bass_utils.run_command = _run_command_dge
    bass_utils._dge_patched = True

F32 = mybir.dt.float32
BF16 = mybir.dt.bfloat16
I16 = mybir.dt.int16
AF = mybir.ActivationFunctionType
ALU = mybir.AluOpType

N_CORES = 8
P = 128
CHUNK_TILES = 8
CHUNK = CHUNK_TILES * P
SPLIT = 32768  # int16 gather-index limit


def _ceil(a, b):
    return -(-a // b)


class Struct:
    pass


# ---------------------------------------------------------------------------
# CPU-side binning (pure numpy)
# ---------------------------------------------------------------------------
def _bin_by_dst(payload, dst, shard, n_blk, half_split):
    """group by (core, dst block); payload split by < SPLIT if asked."""
    out = []
    for c in range(N_CORES):
        lo, hi = c * shard, (c + 1) * shard
        sel = (dst >= lo) & (dst < hi)
        ds = dst[sel] - lo
        ss = payload[sel]
        blk = ds // P
        order = np.argsort(blk, kind="stable")
        ds, ss, blk = ds[order], ss[order], blk[order]
        bounds = np.searchsorted(blk, np.arange(n_blk + 1))
        perblk = []
        for b in range(n_blk):
            sb = ss[bounds[b]:bounds[b + 1]]
            db = ds[bounds[b]:bounds[b + 1]] - b * P
            if half_split:
                m = sb < SPLIT
                perblk.append((sb[m], db[m], sb[~m] - SPLIT, db[~m]))
            else:
                perblk.append((sb, db, np.zeros(0, np.int64), np.zeros(0, np.int64)))
        out.append(perblk)
    return out


def _pack(groups_rc, n_blk, R, half_split):
    """groups_rc[r][c][b] -> common tiles + per-core src16/dstloc arrays."""
    nt = np.zeros((n_blk, R, 2), np.int64)
    for r in range(R):
        for c in range(N_CORES):
            for b in range(n_blk):
                g = groups_rc[r][c][b]
                nt[b, r, 0] = max(nt[b, r, 0], _ceil(len(g[0]), P))
                if half_split:
                    nt[b, r, 1] = max(nt[b, r, 1], _ceil(len(g[2]), P))
    tiles = []
    tmap = {}
    for b in range(n_blk):
        for r in range(R):
            tot = int(nt[b, r, 0] + nt[b, r, 1])
            k = 0
            for half in (0, 1):
                for j in range(int(nt[b, r, half])):
                    tmap[(b, r, half, j)] = len(tiles)
                    tiles.append((b, r, half, k == 0, k == tot - 1))
                    k += 1
    NT = len(tiles)
    src16 = np.zeros((N_CORES, NT, P), np.int16)
    dloc = np.full((N_CORES, NT, P), -1.0, np.float32)
    for c in range(N_CORES):
        for b in range(n_blk):
            for r in range(R):
                g = groups_rc[r][c][b]
                for half in (0, 1):
                    sarr = g[0] if half == 0 else g[2]
                    darr = g[1] if half == 0 else g[3]
                    for j in range(_ceil(len(sarr), P)):
                        t = tmap[(b, r, half, j)]
                        seg_s = sarr[j * P:(j + 1) * P]
                        seg_d = darr[j * P:(j + 1) * P]
                        src16[c, t, :len(seg_s)] = seg_s
                        dloc[c, t, :len(seg_d)] = seg_d
    return tiles, src16, dloc


def _chunks_of(tiles):
    lo = [i for i, t in enumerate(tiles) if t[2] == 0]
    hi = [i for i, t in enumerate(tiles) if t[2] == 1]
    chunks = []
    for half, stream in ((0, lo), (1, hi)):
        for i in range(0, len(stream), CHUNK_TILES):
            chunks.append((half, stream[i:i + CHUNK_TILES]))
    chunks.sort(key=lambda ch: min(ch[1]))
    slot = {}
    for ci, (_, tl) in enumerate(chunks):
        for j, t in enumerate(tl):
            slot[t] = (ci, j)
    return chunks, slot


def _wrap_idx(src16, chunks):
    ncore = src16.shape[0]
    out = np.zeros((ncore, 128, max(1, len(chunks)) * (CHUNK // 16)), np.int16)
    for ci, (_, tl) in enumerate(chunks):
        flat = np.zeros((ncore, CHUNK), np.int16)
        for j, t in enumerate(tl):
            flat[:, j * P:(j + 1) * P] = src16[:, t, :]
        out[:, :16, ci * 64:(ci + 1) * 64] = flat.reshape(
            ncore, CHUNK // 16, 16).transpose(0, 2, 1)
    out[:, 16:, :] = np.tile(out[:, :16, :], (1, 7, 1))
    return out


def prepare(inputs, cfg):
    s = Struct()
    s.cfg = cfg
    N, R, NROW, FK = cfg["N"], cfg["R"], cfg["NROW"], cfg["F"]
    shard, rshard = N // N_CORES, NROW // N_CORES
    n_blk, n_rblk = _ceil(shard, P), _ceil(rshard, P)
    s.shard, s.rshard, s.n_blk, s.n_rblk = shard, rshard, n_blk, n_rblk

    es = np.asarray(inputs["edges_src"]).astype(np.int64)
    ed = np.asarray(inputs["edges_dst"]).astype(np.int64)

    g_main = [_bin_by_dst(es[r], ed[r], shard, n_blk, True) for r in range(R)]
    s.tiles_e, src16_e, dloc_e = _pack(g_main, n_blk, R, True)
    s.chunks_e, s.slot_e = _chunks_of(s.tiles_e)
    s.idx_e = _wrap_idx(src16_e, s.chunks_e)

    g_src = [_bin_by_dst(es[r], es[r], shard, n_blk, False) for r in range(R)]
    s.tiles_s, _, sloc_s = _pack(g_src, n_blk, R, False)

    ridx = np.asarray(inputs["row_idx"]).astype(np.int64)
    rmask = np.asarray(inputs["row_mask"]).astype(bool)
    g_row = [[]]
    for c in range(N_CORES):
        lo = c * rshard
        rows = []
        for bb in range(n_rblk):
            i0 = lo + bb * P
            i1 = min(i0 + P, lo + rshard)
            ii, jj = np.nonzero(rmask[i0:i1])
            srcs = ridx[i0:i1][ii, jj]
            m = srcs < SPLIT
            rows.append((srcs[m], ii[m], srcs[~m] - SPLIT, ii[~m]))
        g_row[0].append(rows)
    s.tiles_r, src16_r, dloc_r = _pack(g_row, n_rblk, 1, True)
    s.chunks_r, s.slot_r = _chunks_of(s.tiles_r)
    s.idx_r = _wrap_idx(src16_r, s.chunks_r)

    s.NT_e, s.NT_s, s.NT_r = len(s.tiles_e), len(s.tiles_s), len(s.tiles_r)
    s.dloc_e = dloc_e.transpose(0, 2, 1)   # [c, 128, NT]
    s.sloc_s = sloc_s.transpose(0, 2, 1)
    s.dloc_r = dloc_r.transpose(0, 2, 1)

    nf = np.asarray(inputs["node_feats"]).astype(np.float32)
    s.nf_shards = [nf[c * shard:(c + 1) * shard] for c in range(N_CORES)]
    mk = rmask.astype(np.float32)
    s.mask_shards = []
    for c in range(N_CORES):
        m = np.zeros((n_rblk * P, FK), np.float32)
        m[:rshard] = mk[c * rshard:(c + 1) * rshard]
        s.mask_shards.append(np.ascontiguousarray(
            m.reshape(n_rblk, P, FK).transpose(1, 0, 2)).reshape(128, -1))
    return s


# ---------------------------------------------------------------------------
# device program
# ---------------------------------------------------------------------------
def build_program(s):
    cfg = s.cfg
    N, R, FK = cfg["N"], cfg["R"], cfg["F"]
    IN_D, HID, NCLS = cfg["IN"], cfg["HID"], cfg["NCLS"]
    TW = 192
    n_blk, n_rblk, shard, rshard = s.n_blk, s.n_rblk, s.shard, s.rshard
    NQ = 4
    NDEG = n_blk * R
    ndeg_tiles = _ceil(2 * NDEG, 128)

    nc = bacc.Bacc("TRN2", target_bir_lowering=False, debug=False,
                   num_devices=N_CORES, num_swdge_queues=NQ)
    dp = nc.declare_dram_parameter
    t_nf = dp("nf", [n_blk * P, IN_D], F32, isOutput=False)
    t_Win = dp("W_in", [IN_D, HID], F32, isOutput=False)
    t_bin = dp("b_in", [1, HID], F32, isOutput=False)
    t_W1 = dp("W1", [R * HID, HID], F32, isOutput=False)
    t_b1 = dp("b1", [R, HID], F32, isOutput=False)
    t_W2 = dp("W2", [R * HID, HID], F32, isOutput=False)
    t_b2 = dp("b2", [R, HID], F32, isOutput=False)
    t_Wm1 = dp("Wm1", [HID, HID], F32, isOutput=False)
    t_bm1 = dp("bm1", [HID, 1], F32, isOutput=False)
    t_Wm2 = dp("Wm2", [HID, HID], F32, isOutput=False)
    t_bm2 = dp("bm2", [HID, 1], F32, isOutput=False)
    t_Wm3 = dp("Wm3", [HID, NCLS], F32, isOutput=False)
    t_bm3 = dp("bm3", [NCLS, 1], F32, isOutput=False)
    t_idx_e = dp("idx_e", list(s.idx_e.shape[1:]), I16, isOutput=False)
    t_idx_r = dp("idx_r", list(s.idx_r.shape[1:]), I16, isOutput=False)
    t_dle = dp("dloc_e", [128, max(1, s.NT_e)], F32, isOutput=False)
    t_dls = dp("sloc_s", [128, max(1, s.NT_s)], F32, isOutput=False)
    t_dlr = dp("dloc_r", [128, max(1, s.NT_r)], F32, isOutput=False)
    t_mask = dp("mask", [128, n_rblk * FK], F32, isOutput=False)
    t_selR = dp("selR", [16, R * 128], F32, isOutput=False)
    t_selB = dp("selB", [max(64, n_rblk), n_rblk * 128], F32, isOutput=False)
    t_out = dp("out", [NCLS, rshard], F32, isOutput=True)

    T = [nc.dram_tensor(f"T{i}", [N, TW], F32) for i in range(3)]
    Tsh = [nc.dram_tensor(f"T{i}sh", [shard, TW], F32) for i in range(3)]

    with tile.TileContext(nc) as tc, ExitStack() as top:
        kp = top.enter_context(tc.tile_pool(name="const", bufs=1))
        wp = top.enter_context(tc.tile_pool(name="weights", bufs=1))
        mp = top.enter_context(tc.tile_pool(name="meta", bufs=1))
        sb = top.enter_context(tc.tile_pool(name="sbwork", bufs=4))
        ttp = top.enter_context(tc.tile_pool(name="ttile", bufs=4))

        iota = kp.tile([128, 128], F32)
        nc.gpsimd.iota(iota[:], pattern=[[1, 128]], base=0, channel_multiplier=0,
                       allow_small_or_imprecise_dtypes=True)
        iota16 = kp.tile([128, 128], BF16)
        nc.gpsimd.iota(iota16[:], pattern=[[1, 128]], base=0, channel_multiplier=0,
                       allow_small_or_imprecise_dtypes=True)
        ident = kp.tile([128, 128], F32)
        make_identity(nc, ident[:])
        ones_col16 = kp.tile([128, 1], BF16)
        nc.vector.memset(ones_col16[:], 1.0)
        ones1 = kp.tile([1, 128], F32)
        nc.vector.memset(ones1[:], 1.0)
        onesR = kp.tile([R, 1], F32)
        nc.vector.memset(onesR[:], 1.0)
        selR = kp.tile([16, R * 128], F32)
        nc.sync.dma_start(out=selR[:], in_=t_selR[:])

        W1sb = wp.tile([128, R * HID], F32)
        W2sb = wp.tile([128, R * HID], F32)
        for r in range(R):
            nc.sync.dma_start(out=W1sb[:HID, r * HID:(r + 1) * HID],
                              in_=t_W1[r * HID:(r + 1) * HID, :])
            nc.sync.dma_start(out=W2sb[:HID, r * HID:(r + 1) * HID],
                              in_=t_W2[r * HID:(r + 1) * HID, :])
        Winsb = wp.tile([IN_D, HID], F32)
        nc.sync.dma_start(out=Winsb[:], in_=t_Win[:])
        Wm1sb = wp.tile([HID, HID], F32)
        nc.sync.dma_start(out=Wm1sb[:], in_=t_Wm1[:])
        Wm2sb = wp.tile([HID, HID], F32)
        nc.sync.dma_start(out=Wm2sb[:], in_=t_Wm2[:])
        Wm3sb = wp.tile([HID, NCLS], F32)
        nc.sync.dma_start(out=Wm3sb[:], in_=t_Wm3[:])
        b1sb = wp.tile([R, HID], F32)
        nc.sync.dma_start(out=b1sb[:], in_=t_b1[:])
        b2sb = wp.tile([R, HID], F32)
        nc.sync.dma_start(out=b2sb[:], in_=t_b2[:])
        binsb = wp.tile([1, HID], F32)
        nc.sync.dma_start(out=binsb[:], in_=t_bin[:])
        bm1sb = wp.tile([HID, 1], F32)
        nc.sync.dma_start(out=bm1sb[:], in_=t_bm1[:])
        bm2sb = wp.tile([HID, 1], F32)
        nc.sync.dma_start(out=bm2sb[:], in_=t_bm2[:])
        bm3sb = wp.tile([NCLS, 1], F32)
        nc.sync.dma_start(out=bm3sb[:], in_=t_bm3[:])

        dle = mp.tile([128, max(1, s.NT_e)], F32)
        nc.sync.dma_start(out=dle[:], in_=t_dle[:])
        dls = mp.tile([128, max(1, s.NT_s)], F32)
        nc.sync.dma_start(out=dls[:], in_=t_dls[:])
        dlr = mp.tile([128, max(1, s.NT_r)], F32)
        nc.sync.dma_start(out=dlr[:], in_=t_dlr[:])
        masksb = mp.tile([128, n_rblk * FK], F32)
        nc.sync.dma_start(out=masksb[:], in_=t_mask[:])

        # ---- replicated biases ------------------------------------------
        binrep = wp.tile([128, HID], F32)
        bsum1 = wp.tile([HID, 1], F32)
        bsum2 = wp.tile([HID, 1], F32)
        with tc.tile_pool(name="ps_init", bufs=1, space="PSUM") as pk:
            ps = pk.tile([128, HID], F32, tag="a")
            nc.tensor.matmul(ps[:], lhsT=ones1[:], rhs=binsb[:], start=True, stop=True)
            nc.vector.tensor_copy(binrep[:], ps[:])
            ps2 = pk.tile([HID, 1], F32, tag="b")
            nc.tensor.matmul(ps2[:], lhsT=b1sb[:], rhs=onesR[:], start=True, stop=True)
            nc.vector.tensor_copy(bsum1[:], ps2[:])
            ps3 = pk.tile([HID, 1], F32, tag="c")
            nc.tensor.matmul(ps3[:], lhsT=b2sb[:], rhs=onesR[:], start=True, stop=True)
            nc.vector.tensor_copy(bsum2[:], ps3[:])

        # ---- phase 1: h0 -------------------------------------------------
        h0sb = mp.tile([128, n_blk * HID], F32)
        with tc.tile_pool(name="ps_h0", bufs=2, space="PSUM") as pp:
            for b in range(n_blk):
                rows = min(P, shard - b * P)
                xt = sb.tile([128, IN_D], F32, tag="xt")
                if rows < P:
                    nc.vector.memset(xt[:], 0.0)
                nc.sync.dma_start(out=xt[:rows, :], in_=t_nf[b * P:b * P + rows, :])
                pst = pp.tile([IN_D, 128], F32, tag="xT")
                nc.tensor.transpose(pst[:], xt[:, :IN_D], ident[:])
                xT = sb.tile([IN_D, 128], F32, tag="xTs")
                nc.vector.tensor_copy(xT[:], pst[:])
                psh = pp.tile([128, HID], F32, tag="h0")
                nc.tensor.matmul(psh[:], lhsT=xT[:], rhs=Winsb[:], start=True,
                                 stop=True)
                tmp = sb.tile([128, HID], F32, tag="h0t")
                nc.vector.tensor_tensor(out=tmp[:], in0=psh[:], in1=binrep[:],
                                        op=ALU.add)
                nc.scalar.activation(h0sb[:, b * HID:(b + 1) * HID], tmp[:], AF.Relu)

        # ---- phase 2: degrees -> norms ----------------------------------
        # per block: psum [128, 16] cols 0:8 deg_in(r), 8:16 deg_out(r)
        e_byblk = {}
        for ti, (b, r, half, first, last) in enumerate(s.tiles_e):
            e_byblk.setdefault(b, []).append((ti, r, last))
        s_byblk = {}
        for ti, (b, r, half, first, last) in enumerate(s.tiles_s):
            s_byblk.setdefault(b, []).append((ti, r, last))
        nscols = mp.tile([128, n_blk * 16], F32)   # n-part norms per block
        normsT = mp.tile([16, n_blk * 128], F32)   # free-dim norms per block
        with (
            tc.tile_pool(name="ps_deg", bufs=2, space="PSUM") as dgp,
            tc.tile_pool(name="ps_degT", bufs=2, space="PSUM") as dtp,
        ):
            for b in range(n_blk):
                degblk = dgp.tile([128, 16], F32, tag="dg")
                nc.vector.memset(degblk[:], 0.0)
                dstarted = set()
                for col_base, byblk, dl in ((0, e_byblk, dle), (8, s_byblk, dls)):
                    for (ti, r, last) in byblk.get(b, []):
                        oh = sb.tile([128, 128], BF16, tag="ohd")
                        nc.vector.tensor_scalar(
                            out=oh[:], in0=iota16[:], scalar1=dl[:, ti:ti + 1],
                            scalar2=None, op0=ALU.is_equal)
                        nc.tensor.matmul(
                            degblk[:, col_base + r:col_base + r + 1],
                            lhsT=oh[:], rhs=ones_col16[:],
                            start=(col_base + r) not in dstarted, stop=last)
                        dstarted.add(col_base + r)
                tmp = sb.tile([128, 16], F32, tag="degt")
                nc.vector.tensor_scalar(out=tmp[:], in0=degblk[:], scalar1=1.0,
                                        scalar2=None, op0=ALU.max)
                tmp2 = sb.tile([128, 16], F32, tag="degt2")
                nc.vector.reciprocal(tmp2[:], tmp[:])
                nc.scalar.activation(nscols[:, b * 16:(b + 1) * 16], tmp2[:],
                                     AF.Sqrt)
                pst = dtp.tile([16, 128], F32, tag="dT")
                nc.tensor.transpose(pst[:], nscols[:, b * 16:(b + 1) * 16],
                                    ident[:])
                nc.vector.tensor_copy(normsT[:, b * 128:(b + 1) * 128], pst[:])

        def write_table(l, b, fill_h):
            rows = min(P, shard - b * P)
            tt = ttp.tile([128, TW], F32, tag="tt")
            nc.vector.memset(tt[:, HID + R:TW], 0.0)
            fill_h(tt)
            nc.vector.tensor_copy(tt[:, HID:HID + R],
                                  nscols[:, b * 16 + 8:b * 16 + 8 + R])
            nc.sync.dma_start(out=Tsh[l][b * P:b * P + rows, :], in_=tt[:rows, :])

        def allgather(l):
            nc.gpsimd.collective_compute(
                "AllGather", ALU.bypass,
                replica_groups=[list(range(N_CORES))],
                ins=[Tsh[l][:]], outs=[T[l][:]])

        for b in range(n_blk):
            write_table(0, b, lambda tt, b=b: nc.vector.tensor_copy(
                tt[:, 0:HID], h0sb[:, b * HID:(b + 1) * HID]))
        allgather(0)

        # ---- phases 3&4: the two RGCN layers ----------------------------
        gp = top.enter_context(tc.tile_pool(name="gather", bufs=6))
        ip = top.enter_context(tc.tile_pool(name="idxt", bufs=4))

        blk_maxr = {}
        for (b, r, half, first, last) in s.tiles_e:
            blk_maxr[b] = max(blk_maxr.get(b, -1), r)

        def run_layer(l):
            Wsb = W1sb if l == 0 else W2sb
            bsum = bsum1 if l == 0 else bsum2
            with (
                tc.tile_pool(name=f"psx{l}", bufs=2, space="PSUM") as psxp,
                tc.tile_pool(name=f"ps2{l}", bufs=1, space="PSUM") as ps2p,
                tc.tile_pool(name=f"aux{l}", bufs=2, space="PSUM") as auxp,
            ):
                gtiles = {}
                for ci, (half, tl) in enumerate(s.chunks_e):
                    it = ip.tile([128, 64], I16, tag="ie")
                    nc.sync.dma_start(out=it[:], in_=t_idx_e[:, ci * 64:(ci + 1) * 64])
                    g = gp.tile([128, CHUNK_TILES, TW], F32, tag="ge")
                    src = T[l][0:SPLIT, :] if half == 0 else T[l][SPLIT:N, :]
                    nc.gpsimd.dma_gather(
                        out_ap=g[:], in_ap=src, idxs_ap=it[:],
                        num_idxs=CHUNK, num_idxs_reg=CHUNK, elem_size=TW,
                        queue_num=ci % NQ)
                    gtiles[ci] = g

                def flush_block(b, psx, started):
                    ps2 = ps2p.tile([128, 128], F32, tag="p2")
                    maxr = blk_maxr[b]
                    any_r = False
                    for r in range(R):
                        if (b, r) not in started:
                            continue
                        pst = auxp.tile([128, 128], F32, tag="ax")
                        nc.tensor.matmul(
                            pst[:], lhsT=selR[:, r * 128:(r + 1) * 128],
                            rhs=normsT[:, b * 128:(b + 1) * 128],
                            start=True, stop=True)
                        ndrep = sb.tile([128, 128], F32, tag="ndr")
                        nc.vector.tensor_copy(ndrep[:], pst[:])
                        xs = sb.tile([128, 128], F32, tag="xs")
                        nc.vector.tensor_tensor(out=xs[:], in0=psx[:, r, :],
                                                in1=ndrep[:], op=ALU.mult)
                        nc.tensor.matmul(ps2[:], lhsT=Wsb[:HID, r * HID:(r + 1) * HID],
                                         rhs=xs[:], start=not any_r,
                                         stop=(r == maxr))
                        any_r = True
                    hsb = sb.tile([128, 128], F32, tag="hsb")
                    if l == 0:
                        nc.scalar.activation(hsb[:], ps2[:], AF.Relu,
                                             bias=bsum[:])
                    else:
                        nc.vector.tensor_scalar(
                            out=hsb[:], in0=ps2[:], scalar1=bsum[:],
                            scalar2=None, op0=ALU.add)
                    pst2 = auxp.tile([128, 128], F32, tag="ax")
                    nc.tensor.transpose(pst2[:], hsb[:], ident[:])
                    write_table(l + 1, b, lambda tt: nc.vector.tensor_copy(
                        tt[:, 0:HID], pst2[:]))

                cur_blk, psx, started = -1, None, set()
                for ti, (b, r, half, first, last) in enumerate(s.tiles_e):
                    if b != cur_blk:
                        if cur_blk >= 0:
                            flush_block(cur_blk, psx, started)
                        cur_blk = b
                        psx = psxp.tile([128, R, 128], F32, tag="psx")
                        started = set()
                    ci, j = s.slot_e[ti]
                    g = gtiles[ci]
                    mg = sb.tile([128, 128], BF16, tag="mg")
                    nc.vector.tensor_scalar(
                        out=mg[:], in0=g[:, j, 0:HID],
                        scalar1=g[:, j, HID + r:HID + r + 1],
                        scalar2=None, op0=ALU.mult)
                    oh = sb.tile([128, 128], BF16, tag="ohm")
                    nc.vector.tensor_scalar(
                        out=oh[:], in0=iota16[:], scalar1=dle[:, ti:ti + 1],
                        scalar2=None, op0=ALU.is_equal)
                    nc.tensor.matmul(psx[:, r, :], lhsT=mg[:], rhs=oh[:],
                                     start=(b, r) not in started, stop=last)
                    started.add((b, r))
                if cur_blk >= 0:
                    flush_block(cur_blk, psx, started)
            allgather(l + 1)

        run_layer(0)
        run_layer(1)

        # ---- phase 5: rows + MLP ----------------------------------------
        NR64 = max(64, n_rblk)
        cnt = mp.tile([128, n_rblk], F32)
        nc.vector.reduce_sum(
            out=cnt[:],
            in_=masksb[:].rearrange("p (b f) -> p b f", f=FK),
            axis=mybir.AxisListType.X)
        cnt2 = mp.tile([128, NR64], F32)
        nc.vector.memset(cnt2[:], 1.0)
        nc.vector.tensor_scalar(out=cnt2[:, :n_rblk], in0=cnt[:], scalar1=1.0,
                                scalar2=None, op0=ALU.max)
        rc = mp.tile([128, NR64], F32)
        nc.vector.reciprocal(rc[:], cnt2[:])
        rcT = mp.tile([NR64, 128], F32)
        with tc.tile_pool(name="ps_rc", bufs=1, space="PSUM") as pp:
            pst = pp.tile([NR64, 128], F32, tag="rcT")
            nc.tensor.transpose(pst[:], rc[:], ident[:])
            nc.vector.tensor_copy(rcT[:], pst[:])
        selB = mp.tile([NR64, n_rblk * 128], F32)
        nc.sync.dma_start(out=selB[:], in_=t_selB[:])

        with (
            tc.tile_pool(name="psr", bufs=2, space="PSUM") as psrp,
            tc.tile_pool(name="psm", bufs=2, space="PSUM") as psmp,
            tc.tile_pool(name="auxr", bufs=2, space="PSUM") as auxp,
        ):
            gtiles = {}
            for ci, (half, tl) in enumerate(s.chunks_r):
                it = ip.tile([128, 64], I16, tag="ir")
                nc.sync.dma_start(out=it[:], in_=t_idx_r[:, ci * 64:(ci + 1) * 64])
                g = gp.tile([128, CHUNK_TILES, TW], F32, tag="ge")
                src = T[2][0:SPLIT, :] if half == 0 else T[2][SPLIT:N, :]
                nc.gpsimd.dma_gather(
                    out_ap=g[:], in_ap=src, idxs_ap=it[:],
                    num_idxs=CHUNK, num_idxs_reg=CHUNK, elem_size=TW,
                    queue_num=ci % NQ)
                gtiles[ci] = g

            def flush_rblock(bb, psr):
                pst = auxp.tile([128, 128], F32, tag="axr")
                nc.tensor.matmul(pst[:], lhsT=selB[:, bb * 128:(bb + 1) * 128],
                                 rhs=rcT[:], start=True, stop=True)
                rrep = sb.tile([128, 128], F32, tag="rrep")
                nc.vector.tensor_copy(rrep[:], pst[:])
                xr = sb.tile([128, 128], F32, tag="xr")
                nc.vector.tensor_tensor(out=xr[:], in0=psr[:], in1=rrep[:],
                                        op=ALU.mult)
                pm = psmp.tile([128, 128], F32, tag="pm")
                nc.tensor.matmul(pm[:], lhsT=Wm1sb[:], rhs=xr[:], start=True,
                                 stop=True)
                a1 = sb.tile([128, 128], F32, tag="a1")
                nc.scalar.activation(a1[:], pm[:], AF.Relu, bias=bm1sb[:])
                pm2 = psmp.tile([128, 128], F32, tag="pm")
                nc.tensor.matmul(pm2[:], lhsT=Wm2sb[:], rhs=a1[:], start=True,
                                 stop=True)
                a2 = sb.tile([128, 128], F32, tag="a2")
                nc.scalar.activation(a2[:], pm2[:], AF.Relu, bias=bm2sb[:])
                pm3 = psmp.tile([NCLS, 128], F32, tag="pm")
                nc.tensor.matmul(pm3[:], lhsT=Wm3sb[:], rhs=a2[:], start=True,
                                 stop=True)
                ot = sb.tile([NCLS, 128], F32, tag="ot")
                nc.vector.tensor_scalar(out=ot[:], in0=pm3[:], scalar1=bm3sb[:],
                                        scalar2=None, op0=ALU.add)
                cols = min(P, rshard - bb * P)
                nc.sync.dma_start(out=t_out[:, bb * P:bb * P + cols],
                                  in_=ot[:, :cols])

            r_byblk = {}
            for ti, (bb, r0, half, first, last) in enumerate(s.tiles_r):
                r_byblk.setdefault(bb, []).append((ti, last))
            for bb in range(n_rblk):
                psr = psrp.tile([128, 128], F32, tag="psrT")
                nc.vector.memset(psr[:], 0.0)
                rstarted = False
                for (ti, last) in r_byblk.get(bb, []):
                    ci, j = s.slot_r[ti]
                    g = gtiles[ci]
                    mg = sb.tile([128, 128], BF16, tag="mg")
                    nc.vector.tensor_copy(mg[:], g[:, j, 0:HID])
                    oh = sb.tile([128, 128], BF16, tag="ohm")
                    nc.vector.tensor_scalar(
                        out=oh[:], in0=iota16[:], scalar1=dlr[:, ti:ti + 1],
                        scalar2=None, op0=ALU.is_equal)
                    nc.tensor.matmul(psr[:], lhsT=mg[:], rhs=oh[:],
                                     start=not rstarted, stop=last)
                    rstarted = True
                flush_rblock(bb, psr)

    nc.compile()
    return nc


# ---------------------------------------------------------------------------
# entry point
# ---------------------------------------------------------------------------
def _selR(cfg):
    R = cfg["R"]
    a = np.zeros((16, R * 128), np.float32)
    for r in range(R):
        a[r, r * 128:(r + 1) * 128] = 1.0
    return a


def _selB(s):
    a = np.zeros((max(64, s.n_rblk), s.n_rblk * 128), np.float32)
    for bb in range(s.n_rblk):
        a[bb, bb * 128:(bb + 1) * 128] = 1.0
    return a


def run(inputs, cfg):
    s = prepare(inputs, cfg)
    nc = build_program(s)
    in_maps = []
    for c in range(N_CORES):
        nfp = np.zeros((s.n_blk * P, cfg["IN"]), np.float32)
        nfp[:s.shard] = s.nf_shards[c]
        m = {
            "nf": nfp,
            "W_in": np.asarray(inputs["W_in"], np.float32),
            "b_in": np.asarray(inputs["b_in"], np.float32).reshape(1, -1),
            "W1": np.asarray(inputs["W1"], np.float32).reshape(-1, cfg["HID"]),
            "b1": np.asarray(inputs["b1"], np.float32),
            "W2": np.asarray(inputs["W2"], np.float32).reshape(-1, cfg["HID"]),
            "b2": np.asarray(inputs["b2"], np.float32),
            "Wm1": np.asarray(inputs["Wm1"], np.float32),
            "bm1": np.asarray(inputs["bm1"], np.float32).reshape(-1, 1),
            "Wm2": np.asarray(inputs["Wm2"], np.float32),
            "bm2": np.asarray(inputs["bm2"], np.float32).reshape(-1, 1),
            "Wm3": np.asarray(inputs["Wm3"], np.float32),
            "bm3": np.asarray(inputs["bm3"], np.float32).reshape(-1, 1),
            "idx_e": np.ascontiguousarray(s.idx_e[c]),
            "idx_r": np.ascontiguousarray(s.idx_r[c]),
            "dloc_e": np.ascontiguousarray(s.dloc_e[c]),
            "sloc_s": np.ascontiguousarray(s.sloc_s[c]),
            "dloc_r": np.ascontiguousarray(s.dloc_r[c]),
            "mask": np.ascontiguousarray(s.mask_shards[c]),
            "selR": _selR(cfg),
            "selB": _selB(s),
        }
        in_maps.append(m)
    res = bass_utils.run_bass_kernel_spmd(nc, in_maps,
                                          core_ids=list(range(N_CORES)))
    out = np.concatenate(
        [res.results[c]["out"][:, :s.rshard].T for c in range(N_CORES)], axis=0)
    return out.astype(np.float32), s, nc, in_maps


def kernel(node_feats, edges_src, edges_dst, row_idx, row_mask,
           W_in, b_in, W1, b1, W2, b2, Wm1, bm1, Wm2, bm2, Wm3, bm3):
    cfg = dict(N=38000, R=8, NROW=60000, F=19, IN=64, HID=128, NCLS=10)
    inputs = dict(node_feats=node_feats, edges_src=edges_src,
                  edges_dst=edges_dst, row_idx=row_idx, row_mask=row_mask,
                  W_in=W_in, b_in=b_in, W1=W1, b1=b1, W2=W2, b2=b2,
                  Wm1=Wm1, bm1=bm1, Wm2=Wm2, bm2=bm2, Wm3=Wm3, bm3=bm3)
    out, _, _, _ = run(inputs, cfg)
    return out
